# revision 1
# baseline (speedup 1.0000x reference)
"""Bass/Trainium2 kernel for nn_MD_LSTM (2-layer LSTM encoder + autoregressive decoder).

Strategy: tensor-parallel over the 4H gate dimension across 8 cores (each core
owns 128 hidden units per layer = 512 gate rows). All weights SBUF-resident in
float32r (11-bit-mantissa fp32, runs at bf16 PE speed for N>=256). States kept
transposed [H, B] so B=256 is the matmul moving dim. Per step, each core's
h-slice [128, B] is AllGathered; gather windows are filled with recurrent-weight
matmuls that only need previous-step state.
"""
import sys
sys.path.insert(0, "/opt/trn_rl_repo")
import numpy as np

P = 128
B, S, F, H = 256, 100, 256, 1024
NCORES = 8
NG = 4
NKH = H // P          # 8
NKF = F // P          # 2
NFT = F // P          # 2

_CACHE = {}


def _build(n_enc, n_dec, dbg=False):
    import concourse.bass as bass
    import concourse.tile as tile
    from concourse import bacc, mybir

    dt = mybir.dt
    AF = mybir.ActivationFunctionType
    f32, f32r = dt.float32, dt.float32r

    nc = bacc.Bacc("TRN2", target_bir_lowering=False, debug=False,
                   enable_asserts=True, num_devices=NCORES)

    # ---- I/O ----
    d_xT = nc.dram_tensor("xT", [NKF, P, n_enc * B], f32, kind="ExternalInput")
    d_wih0 = nc.dram_tensor("wih0t", [P, NKF, NG, P], f32, kind="ExternalInput")
    d_whh0 = nc.dram_tensor("whh0t", [P, NKH, NG, P], f32, kind="ExternalInput")
    d_wih1 = nc.dram_tensor("wih1t", [P, NKH, NG, P], f32, kind="ExternalInput")
    d_whh1 = nc.dram_tensor("whh1t", [P, NKH, NG, P], f32, kind="ExternalInput")
    d_outw = nc.dram_tensor("outwt", [P, NKH, NFT, P], f32, kind="ExternalInput")
    d_outwd = nc.dram_tensor("outwtd", [P, NKH, NFT, P], f32, kind="ExternalInput")
    d_b0 = nc.dram_tensor("b0t", [P, NG], f32, kind="ExternalInput")
    d_b1 = nc.dram_tensor("b1t", [P, NG], f32, kind="ExternalInput")
    d_outb = nc.dram_tensor("outbt", [P, NFT], f32, kind="ExternalInput")
    d_cdec = nc.dram_tensor("cdect", [P, NFT], f32, kind="ExternalInput")
    d_enc = nc.dram_tensor("enc_out", [n_enc, NFT, P, B], f32, kind="ExternalOutput")
    if dbg:
        d_h0g = nc.dram_tensor("h0g_dbg", [P, NKH, B], f32, kind="ExternalOutput")
        d_h1g = nc.dram_tensor("h1g_dbg", [P, NKH, B], f32, kind="ExternalOutput")
        d_g0 = nc.dram_tensor("g0_dbg", [P, 4 * B], f32, kind="ExternalOutput")
        d_xw = nc.dram_tensor("xw_dbg", [P, NG, B], f32, kind="ExternalOutput")
    d_dec = nc.dram_tensor("dec_out", [n_dec, NFT, P, B], f32, kind="ExternalOutput")

    # internal DRAM
    d_xw0 = nc.dram_tensor("xw0s", [n_enc, NG, P, B], f32)
    d_cc0i = nc.dram_tensor("cc0i", [P, B], f32)
    d_cc0o = nc.dram_tensor("cc0o", [NCORES * P, B], f32)
    d_cc1i = nc.dram_tensor("cc1i", [P, B], f32)
    d_cc1o = nc.dram_tensor("cc1o", [NCORES * P, B], f32)

    rg = [list(range(NCORES))]

    with tile.TileContext(nc) as tc:
        with (
            tc.tile_pool(name="wp", bufs=1) as wp,        # persistent weights/state
            tc.tile_pool(name="xc", bufs=3) as xcp,       # bulk x chunks
            tc.tile_pool(name="bo", bufs=3) as bop,       # bulk psum->sbuf staging
            tc.tile_pool(name="xw", bufs=3) as xwp,       # per-step xw0
            tc.tile_pool(name="xt", bufs=3) as xtp,       # per-step xT slice
            tc.tile_pool(name="pw", bufs=3) as pwp,       # pointwise temps
            tc.tile_pool(name="hs", bufs=2) as hsp,       # h send tiles
            tc.tile_pool(name="eo", bufs=3) as eop,       # enc out staging
            tc.tile_pool(name="nt", bufs=3) as ntp,       # dec new staging
            tc.tile_pool(name="lt", bufs=2) as ltp,       # lastT ping-pong
            tc.tile_pool(name="pg", bufs=1, space="PSUM") as pg,
        ):
            # ---- persistent tiles ----
            w_ih0 = wp.tile([P, NKF, NG, P], f32r)
            w_hh0 = wp.tile([P, NKH, NG, P], f32r)
            w_ih1 = wp.tile([P, NKH, NG, P], f32r)
            w_hh1 = wp.tile([P, NKH, NG, P], f32r)
            w_out = wp.tile([P, NKH, NFT, P], f32r)
            w_outd = wp.tile([P, NKH, NFT, P], f32r)
            nc.gpsimd.dma_start(w_ih0[:], d_wih0[:])
            nc.gpsimd.dma_start(w_hh0[:], d_whh0[:])
            nc.gpsimd.dma_start(w_ih1[:], d_wih1[:])
            nc.gpsimd.dma_start(w_hh1[:], d_whh1[:])
            nc.gpsimd.dma_start(w_out[:], d_outw[:])
            nc.gpsimd.dma_start(w_outd[:], d_outwd[:])
            sb_b0 = wp.tile([P, NG], f32)
            sb_b1 = wp.tile([P, NG], f32)
            sb_outb = wp.tile([P, NFT], f32)
            sb_cdec = wp.tile([P, NFT], f32)
            nc.sync.dma_start(sb_b0[:], d_b0[:])
            nc.sync.dma_start(sb_b1[:], d_b1[:])
            nc.sync.dma_start(sb_outb[:], d_outb[:])
            nc.sync.dma_start(sb_cdec[:], d_cdec[:])

            h0g = wp.tile([P, NKH, B], f32r)   # gathered h0, [p, k, b]
            h1g = wp.tile([P, NKH, B], f32r)
            c0 = wp.tile([P, B], f32)
            c1 = wp.tile([P, B], f32)
            nc.gpsimd.memset(h0g[:].bitcast(f32), 0.0)
            nc.gpsimd.memset(h1g[:].bitcast(f32), 0.0)
            nc.gpsimd.memset(c0[:], 0.0)
            nc.gpsimd.memset(c1[:], 0.0)

            # ---- bulk precompute: xw0[s] = W_ih0_k @ xT(s) for all encoder steps ----
            CHW = 512
            steps_per_chunk = CHW // B  # 2
            n_chunks = n_enc * B // CHW
            for c in range(n_chunks):
                xc = xcp.tile([P, NKF, CHW], f32r, name="xchunk")
                nc.gpsimd.dma_start(
                    xc[:], d_xT[:, :, c * CHW:(c + 1) * CHW].rearrange("a p n -> p a n"))
                for g in range(NG):
                    pb = pg.tile([P, CHW], f32, name="pj")
                    for kf in range(NKF):
                        nc.tensor.matmul(pb[:], w_ih0[:, kf, g, :], xc[:, kf, :],
                                         start=(kf == 0), stop=(kf == NKF - 1))
                    so = bop.tile([P, CHW], f32, name="bstage")
                    nc.vector.tensor_copy(so[:], pb[:])
                    nc.sync.dma_start(
                        d_xw0[c * steps_per_chunk:(c + 1) * steps_per_chunk, g]
                        .rearrange("s p b -> p s b"),
                        so.rearrange("p (s b) -> p s b", s=steps_per_chunk))

            ACT_SIG, ACT_TANH = AF.Sigmoid, AF.Tanh

            def gate_pointwise(psA, psB, bias_sb, cstate, hout, pre_add=None):
                """LSTM cell pointwise from gate psums (i,f in psA halves; g,o in psB).
                pre_add: optional sbuf [P, NG, B] added to psums first (encoder xw0)."""
                acts = []
                for g, fn in ((0, ACT_SIG), (1, ACT_SIG), (2, ACT_TANH), (3, ACT_SIG)):
                    ps = (psA, psB)[g // 2][:, (g % 2) * B:(g % 2) * B + B]
                    a = pwp.tile([P, B], f32, name=f"act{g}")
                    if pre_add is not None:
                        pre = pwp.tile([P, B], f32, name=f"pre{g}")
                        nc.vector.tensor_add(pre[:], ps, pre_add[:, g, :])
                        nc.scalar.activation(a[:], pre[:], fn, bias=bias_sb[:, g:g + 1])
                    else:
                        nc.scalar.activation(a[:], ps, fn, bias=bias_sb[:, g:g + 1])
                    acts.append(a)
                si, sf, tg, so_ = acts
                t1 = pwp.tile([P, B], f32, name="t1")
                t2 = pwp.tile([P, B], f32, name="t2")
                nc.vector.tensor_mul(t1[:], sf[:], cstate[:])
                nc.vector.tensor_mul(t2[:], si[:], tg[:])
                nc.vector.tensor_add(cstate[:], t1[:], t2[:])
                tcs = pwp.tile([P, B], f32, name="tc")
                nc.scalar.activation(tcs[:], cstate[:], ACT_TANH)
                nc.vector.tensor_mul(hout[:], so_[:], tcs[:])

            def allgather(hsend, cci, cco, hg):
                nc.sync.dma_start(cci[:], hsend[:])
                nc.gpsimd.collective_compute(
                    "AllGather", mybir.AluOpType.bypass, replica_groups=rg,
                    ins=[cci.ap().opt()], outs=[cco.ap().opt()])
                nc.gpsimd.dma_start(hg[:], cco.ap().rearrange("(o p) b -> p o b", p=P))

            def emit_enc_proj(t):
                """dx(t) projection + encoder output; returns staging tile."""
                pj = pg.tile([P, NFT * B], f32, name="pj")
                for ft in range(NFT):
                    for kh in range(NKH):
                        nc.tensor.matmul(pj[:, ft * B:(ft + 1) * B],
                                         w_out[:, kh, ft, :], h1g[:, kh, :],
                                         start=(kh == 0), stop=(kh == NKH - 1))
                xts = xtp.tile([P, NFT, B], f32, name="xts")
                nc.sync.dma_start(
                    xts[:], d_xT[:, :, t * B:(t + 1) * B].rearrange("a p b -> p a b"))
                eo = eop.tile([P, NFT, B], f32, name="eo")
                for ft in range(NFT):
                    tmp = pwp.tile([P, B], f32, name="ptmp")
                    nc.vector.tensor_scalar_add(tmp[:], pj[:, ft * B:(ft + 1) * B],
                                                sb_outb[:, ft:ft + 1])
                    nc.vector.tensor_add(eo[:, ft, :], tmp[:], xts[:, ft, :])
                nc.sync.dma_start(d_enc[t].rearrange("f p b -> p f b"), eo[:])
                return eo

            # ---- encoder ----
            for t in range(n_enc):
                psA = pg.tile([P, 2 * B], f32, name="g0a")
                psB = pg.tile([P, 2 * B], f32, name="g0b")
                for g in range(NG):
                    ps = (psA, psB)[g // 2][:, (g % 2) * B:(g % 2) * B + B]
                    for kh in range(NKH):
                        nc.tensor.matmul(ps, w_hh0[:, kh, g, :], h0g[:, kh, :],
                                         start=(kh == 0), stop=(kh == NKH - 1))
                if t > 0:
                    emit_enc_proj(t - 1)
                xw = xwp.tile([P, NG, B], f32, name="xw")
                nc.sync.dma_start(xw[:], d_xw0[t].rearrange("g p b -> p g b"))
                h0send = hsp.tile([P, B], f32, name="h0send")
                gate_pointwise(psA, psB, sb_b0, c0, h0send, pre_add=xw)
                allgather(h0send, d_cc0i, d_cc0o, h0g)

                psC = pg.tile([P, 2 * B], f32, name="g1a")
                psD = pg.tile([P, 2 * B], f32, name="g1b")
                for g in range(NG):
                    ps = (psC, psD)[g // 2][:, (g % 2) * B:(g % 2) * B + B]
                    for kh in range(NKH):
                        nc.tensor.matmul(ps, w_hh1[:, kh, g, :], h1g[:, kh, :],
                                         start=(kh == 0), stop=False)
                    for kh in range(NKH):
                        nc.tensor.matmul(ps, w_ih1[:, kh, g, :], h0g[:, kh, :],
                                         start=False, stop=(kh == NKH - 1))
                h1send = hsp.tile([P, B], f32, name="h1send")
                gate_pointwise(psC, psD, sb_b1, c1, h1send)
                allgather(h1send, d_cc1i, d_cc1o, h1g)
                if dbg and t == 1:
                    nc.sync.dma_start(d_h0g[:], h0g[:].bitcast(f32))
                    nc.sync.dma_start(d_h1g[:], h1g[:].bitcast(f32))
                    nc.sync.dma_start(d_g0[:], d_g0[:])  # placeholder no-op read
                    nc.sync.dma_start(d_xw[:], xw[:])

            eo_last = emit_enc_proj(n_enc - 1)
            lastT = ltp.tile([P, NFT, B], f32r, name="lastT")
            nc.gpsimd.dma_start(lastT[:], eo_last[:])

            def emit_dec_proj(j):
                """new(j) = last + outWd @ h1(j) + cdec; updates lastT, writes dec_out[j]."""
                nonlocal lastT
                pj = pg.tile([P, NFT * B], f32, name="pj")
                for ft in range(NFT):
                    for kh in range(NKH):
                        nc.tensor.matmul(pj[:, ft * B:(ft + 1) * B],
                                         w_outd[:, kh, ft, :], h1g[:, kh, :],
                                         start=(kh == 0), stop=(kh == NKH - 1))
                newt = ntp.tile([P, NFT, B], f32, name="newt")
                for ft in range(NFT):
                    tmp = pwp.tile([P, B], f32, name="ptmp")
                    nc.vector.tensor_scalar_add(tmp[:], pj[:, ft * B:(ft + 1) * B],
                                                sb_cdec[:, ft:ft + 1])
                    nc.vector.tensor_add(newt[:, ft, :], tmp[:],
                                         lastT[:, ft, :].bitcast(f32))
                nc.sync.dma_start(d_dec[j].rearrange("f p b -> p f b"), newt[:])
                lt_new = ltp.tile([P, NFT, B], f32r, name="lastT")
                nc.gpsimd.dma_start(lt_new[:], newt[:])
                lastT = lt_new

            # ---- decoder ----
            for j in range(n_dec):
                if j > 0:
                    emit_dec_proj(j - 1)
                psA = pg.tile([P, 2 * B], f32, name="g0a")
                psB = pg.tile([P, 2 * B], f32, name="g0b")
                for g in range(NG):
                    ps = (psA, psB)[g // 2][:, (g % 2) * B:(g % 2) * B + B]
                    for kh in range(NKH):
                        nc.tensor.matmul(ps, w_hh0[:, kh, g, :], h0g[:, kh, :],
                                         start=(kh == 0), stop=False)
                    for kf in range(NKF):
                        nc.tensor.matmul(ps, w_ih0[:, kf, g, :], lastT[:, kf, :],
                                         start=False, stop=(kf == NKF - 1))
                h0send = hsp.tile([P, B], f32, name="h0send")
                gate_pointwise(psA, psB, sb_b0, c0, h0send)
                allgather(h0send, d_cc0i, d_cc0o, h0g)

                psC = pg.tile([P, 2 * B], f32, name="g1a")
                psD = pg.tile([P, 2 * B], f32, name="g1b")
                for g in range(NG):
                    ps = (psC, psD)[g // 2][:, (g % 2) * B:(g % 2) * B + B]
                    for kh in range(NKH):
                        nc.tensor.matmul(ps, w_hh1[:, kh, g, :], h1g[:, kh, :],
                                         start=(kh == 0), stop=False)
                    for kh in range(NKH):
                        nc.tensor.matmul(ps, w_ih1[:, kh, g, :], h0g[:, kh, :],
                                         start=False, stop=(kh == NKH - 1))
                h1send = hsp.tile([P, B], f32, name="h1send")
                gate_pointwise(psC, psD, sb_b1, c1, h1send)
                allgather(h1send, d_cc1i, d_cc1o, h1g)

            emit_dec_proj(n_dec - 1)

    nc.compile()
    return nc


def _prep_inputs(x, W_ih0, W_hh0, b0, W_ih1, W_hh1, b1, out_W, out_b, dy_mu, dy_std,
                 n_enc):
    """Host-side transposes/shards. Returns list of per-core input dicts."""
    xT = np.ascontiguousarray(x[:, :n_enc].transpose(2, 1, 0)).reshape(NKF, P, n_enc * B)

    def gate_slices(W, k, nk):
        # W [4H, K] -> per-core slice, laid out [P, nk, NG, P]
        w4 = W.reshape(NG, H, W.shape[1])[:, k * P:(k + 1) * P, :]     # [NG, P(m), K]
        return np.ascontiguousarray(
            w4.reshape(NG, P, nk, P).transpose(3, 2, 0, 1))            # [P, nk, NG, P]

    out_Wd = out_W * dy_std[:, None]
    outwt = np.ascontiguousarray(out_W.reshape(NFT, P, NKH, P).transpose(3, 2, 0, 1))
    outwtd = np.ascontiguousarray(out_Wd.reshape(NFT, P, NKH, P).transpose(3, 2, 0, 1))
    outbt = np.ascontiguousarray(out_b.reshape(NFT, P).T)
    cdect = np.ascontiguousarray((dy_std * out_b + dy_mu).reshape(NFT, P).T)

    in_maps = []
    for k in range(NCORES):
        b0t = np.ascontiguousarray(b0.reshape(NG, H)[:, k * P:(k + 1) * P].T)
        b1t = np.ascontiguousarray(b1.reshape(NG, H)[:, k * P:(k + 1) * P].T)
        in_maps.append({
            "xT": xT,
            "wih0t": gate_slices(W_ih0, k, NKF),
            "whh0t": gate_slices(W_hh0, k, NKH),
            "wih1t": gate_slices(W_ih1, k, NKH),
            "whh1t": gate_slices(W_hh1, k, NKH),
            "outwt": outwt, "outwtd": outwtd,
            "b0t": b0t, "b1t": b1t, "outbt": outbt, "cdect": cdect,
        })
    return in_maps


def run_device(x, W_ih0, W_hh0, b0, W_ih1, W_hh1, b1, out_W, out_b, dy_mu, dy_std,
               n_enc, n_dec):
    """Run the Bass kernel; returns (enc_out, dec_out) from core 0."""
    from concourse.bass_utils import run_bass_kernel_spmd
    key = (n_enc, n_dec)
    if key not in _CACHE:
        _CACHE[key] = _build(n_enc, n_dec)
    nc = _CACHE[key]
    in_maps = _prep_inputs(x, W_ih0, W_hh0, b0, W_ih1, W_hh1, b1, out_W, out_b,
                           dy_mu, dy_std, n_enc)
    res = run_bass_kernel_spmd(nc, in_maps, core_ids=list(range(NCORES)), trace=False)
    out = res.results[0]
    return out["enc_out"], out["dec_out"]


def kernel(**inputs):
    x = np.asarray(inputs["x"], np.float32)
    t = int(np.asarray(inputs["t"]))
    args = [np.asarray(inputs[k], np.float32) for k in
            ["W_ih0", "W_hh0", "b0", "W_ih1", "W_hh1", "b1",
             "out_W", "out_b", "dy_mu", "dy_std"]]
    n_enc, n_dec = x.shape[1], t - 1
    enc, dec = run_device(x, *args, n_enc, n_dec)
    # enc [S, NFT, P, B] -> [B, S, F]; same for dec
    enc_b = np.ascontiguousarray(enc.transpose(3, 0, 1, 2)).reshape(B, n_enc, F)
    dec_b = np.ascontiguousarray(dec.transpose(3, 0, 1, 2)).reshape(B, n_dec, F)
    return np.concatenate([x[:, :1], enc_b, dec_b], axis=1)



# revision 2
# speedup vs baseline: 1.0334x; 1.0334x over previous
"""Bass/Trainium2 kernel for nn_MD_LSTM (2-layer LSTM encoder + autoregressive decoder).

Strategy: tensor-parallel over the 4H gate dimension across 8 cores (each core
owns 128 hidden units per layer = 512 gate rows). Recurrent/gathered-h weights
are SBUF-resident bf16 (full PE rate at any moving size and p-state); the
x/lastT path stays float32r. States kept transposed [H, B] so B=256 is the
matmul moving dim. Per step, each core's h-slice [128, B] is AllGathered in
bf16; gather loads are split so the first K-chunks land early. The decoder is
software-pipelined: the next step's Whh0 matmuls (one PSUM bank per gate, one
pending accumulation group each) issue before the output projection so they
fill the AG(h1) wait, and Wih0@lastT closes the groups after the projection
updates lastT.
"""
import sys
sys.path.insert(0, "/opt/trn_rl_repo")
import numpy as np

P = 128
B, S, F, H = 256, 100, 256, 1024
NCORES = 8
NG = 4
NKH = H // P          # 8
NKF = F // P          # 2
NFT = F // P          # 2

_CACHE = {}


def _build(n_enc, n_dec):
    import concourse.tile as tile
    from concourse import bacc, mybir

    dt = mybir.dt
    AF = mybir.ActivationFunctionType
    f32, f32r, bf = dt.float32, dt.float32r, dt.bfloat16

    nc = bacc.Bacc("TRN2", target_bir_lowering=False, debug=False,
                   enable_asserts=True, num_devices=NCORES)

    # ---- I/O ----
    d_xT = nc.dram_tensor("xT", [NKF, P, n_enc * B], f32, kind="ExternalInput")
    d_wih0 = nc.dram_tensor("wih0t", [P, NKF, NG, P], f32, kind="ExternalInput")
    d_whh0 = nc.dram_tensor("whh0t", [P, NKH, NG, P], bf, kind="ExternalInput")
    d_wih1 = nc.dram_tensor("wih1t", [P, NKH, NG, P], bf, kind="ExternalInput")
    d_whh1 = nc.dram_tensor("whh1t", [P, NKH, NG, P], bf, kind="ExternalInput")
    d_outw = nc.dram_tensor("outwt", [P, NKH, NFT, P], bf, kind="ExternalInput")
    d_outwd = nc.dram_tensor("outwtd", [P, NKH, NFT, P], bf, kind="ExternalInput")
    d_b0 = nc.dram_tensor("b0t", [P, NG], f32, kind="ExternalInput")
    d_b1 = nc.dram_tensor("b1t", [P, NG], f32, kind="ExternalInput")
    d_outb = nc.dram_tensor("outbt", [P, NFT], f32, kind="ExternalInput")
    d_cdec = nc.dram_tensor("cdect", [P, NFT], f32, kind="ExternalInput")
    d_enc = nc.dram_tensor("enc_out", [n_enc, NFT, P, B], f32, kind="ExternalOutput")
    d_dec = nc.dram_tensor("dec_out", [n_dec, NFT, P, B], f32, kind="ExternalOutput")

    # internal DRAM (AllGather bounce buffers, bf16 payload)
    d_xw0 = nc.dram_tensor("xw0s", [n_enc, NG, P, B], f32)
    d_cc0i = nc.dram_tensor("cc0i", [P, B], bf)
    d_cc0o = nc.dram_tensor("cc0o", [NCORES * P, B], bf)
    d_cc1i = nc.dram_tensor("cc1i", [P, B], bf)
    d_cc1o = nc.dram_tensor("cc1o", [NCORES * P, B], bf)

    rg = [list(range(NCORES))]

    with tile.TileContext(nc) as tc:
        with (
            tc.tile_pool(name="wp", bufs=1) as wp,        # persistent weights/state
            tc.tile_pool(name="xc", bufs=3) as xcp,       # bulk x chunks
            tc.tile_pool(name="bo", bufs=3) as bop,       # bulk psum->sbuf staging
            tc.tile_pool(name="xw", bufs=3) as xwp,       # per-step xw0
            tc.tile_pool(name="xt", bufs=3) as xtp,       # per-step xT slice
            tc.tile_pool(name="pw", bufs=3) as pwp,       # pointwise temps
            tc.tile_pool(name="hs", bufs=2) as hsp,       # h send tiles
            tc.tile_pool(name="eo", bufs=3) as eop,       # enc out staging
            tc.tile_pool(name="nt", bufs=3) as ntp,       # dec new staging
            tc.tile_pool(name="lt", bufs=2) as ltp,       # lastT ping-pong
            tc.tile_pool(name="pg", bufs=1, space="PSUM") as pg,    # gates0 banks
            tc.tile_pool(name="pg2", bufs=1, space="PSUM") as pg2,  # gates1 banks
            tc.tile_pool(name="pg3", bufs=1, space="PSUM") as pg3,  # proj/bulk bank
        ):
            # ---- persistent tiles ----
            w_ih0 = wp.tile([P, NKF, NG, P], f32r)
            w_hh0 = wp.tile([P, NKH, NG, P], bf)
            w_ih1 = wp.tile([P, NKH, NG, P], bf)
            w_hh1 = wp.tile([P, NKH, NG, P], bf)
            w_out = wp.tile([P, NKH, NFT, P], bf)
            w_outd = wp.tile([P, NKH, NFT, P], bf)
            nc.gpsimd.dma_start(w_ih0[:], d_wih0[:])
            nc.gpsimd.dma_start(w_hh0[:], d_whh0[:])
            nc.gpsimd.dma_start(w_ih1[:], d_wih1[:])
            nc.gpsimd.dma_start(w_hh1[:], d_whh1[:])
            nc.gpsimd.dma_start(w_out[:], d_outw[:])
            nc.gpsimd.dma_start(w_outd[:], d_outwd[:])
            sb_b0 = wp.tile([P, NG], f32)
            sb_b1 = wp.tile([P, NG], f32)
            sb_outb = wp.tile([P, NFT], f32)
            sb_cdec = wp.tile([P, NFT], f32)
            nc.sync.dma_start(sb_b0[:], d_b0[:])
            nc.sync.dma_start(sb_b1[:], d_b1[:])
            nc.sync.dma_start(sb_outb[:], d_outb[:])
            nc.sync.dma_start(sb_cdec[:], d_cdec[:])

            h0g = wp.tile([P, NKH, B], bf)   # gathered h0, [p, k, b]
            h1g = wp.tile([P, NKH, B], bf)
            c0 = wp.tile([P, B], f32)
            c1 = wp.tile([P, B], f32)
            nc.gpsimd.memset(h0g[:], 0.0)
            nc.gpsimd.memset(h1g[:], 0.0)
            nc.gpsimd.memset(c0[:], 0.0)
            nc.gpsimd.memset(c1[:], 0.0)

            # ---- bulk precompute: xw0[s] = W_ih0_k @ xT(s) for all encoder steps ----
            CHW = 512
            steps_per_chunk = CHW // B  # 2
            n_chunks = n_enc * B // CHW
            for c in range(n_chunks):
                xc = xcp.tile([P, NKF, CHW], f32r, name="xchunk")
                nc.gpsimd.dma_start(
                    xc[:], d_xT[:, :, c * CHW:(c + 1) * CHW].rearrange("a p n -> p a n"))
                for g in range(NG):
                    pb = pg3.tile([P, CHW], f32, name="pj")
                    for kf in range(NKF):
                        nc.tensor.matmul(pb[:], w_ih0[:, kf, g, :], xc[:, kf, :],
                                         start=(kf == 0), stop=(kf == NKF - 1))
                    so = bop.tile([P, CHW], f32, name="bstage")
                    nc.vector.tensor_copy(so[:], pb[:])
                    nc.sync.dma_start(
                        d_xw0[c * steps_per_chunk:(c + 1) * steps_per_chunk, g]
                        .rearrange("s p b -> p s b"),
                        so.rearrange("p (s b) -> p s b", s=steps_per_chunk))

            ACT_SIG, ACT_TANH = AF.Sigmoid, AF.Tanh

            def gate_pointwise(psA, psB, bias_sb, cstate, hout, pre_add=None):
                """LSTM cell pointwise from gate psums (i,f in psA halves; g,o in psB).
                pre_add: optional sbuf [P, NG, B] added to psums first (encoder xw0)."""
                acts = []
                for g, fn in ((0, ACT_SIG), (1, ACT_SIG), (2, ACT_TANH), (3, ACT_SIG)):
                    ps = (psA, psB)[g // 2][:, (g % 2) * B:(g % 2) * B + B]
                    a = pwp.tile([P, B], f32, name=f"act{g}")
                    if pre_add is not None:
                        pre = pwp.tile([P, B], f32, name=f"pre{g}")
                        nc.vector.tensor_add(pre[:], ps, pre_add[:, g, :])
                        nc.scalar.activation(a[:], pre[:], fn, bias=bias_sb[:, g:g + 1])
                    else:
                        nc.scalar.activation(a[:], ps, fn, bias=bias_sb[:, g:g + 1])
                    acts.append(a)
                si, sf, tg, so_ = acts
                t1 = pwp.tile([P, B], f32, name="t1")
                t2 = pwp.tile([P, B], f32, name="t2")
                nc.vector.tensor_mul(t1[:], sf[:], cstate[:])
                nc.vector.tensor_mul(t2[:], si[:], tg[:])
                nc.vector.tensor_add(cstate[:], t1[:], t2[:])
                tcs = pwp.tile([P, B], f32, name="tc")
                nc.scalar.activation(tcs[:], cstate[:], ACT_TANH)
                nc.vector.tensor_mul(hout[:], so_[:], tcs[:])

            def allgather(hsend, cci, cco, hg):
                nc.sync.dma_start(cci[:], hsend[:])
                nc.gpsimd.collective_compute(
                    "AllGather", mybir.AluOpType.bypass, replica_groups=rg,
                    ins=[cci.ap().opt()], outs=[cco.ap().opt()])
                # split the gather load so the first K-chunks land earlier
                half = NKH // 2
                cr = cco.ap().rearrange("(o p) b -> p o b", p=P)
                nc.gpsimd.dma_start(hg[:, :half, :], cr[:, :half, :])
                nc.gpsimd.dma_start(hg[:, half:, :], cr[:, half:, :])

            def emit_enc_proj(t):
                """dx(t) projection + encoder output; returns staging tile."""
                pj = pg3.tile([P, NFT * B], f32, name="pj")
                for ft in range(NFT):
                    for kh in range(NKH):
                        nc.tensor.matmul(pj[:, ft * B:(ft + 1) * B],
                                         w_out[:, kh, ft, :], h1g[:, kh, :],
                                         start=(kh == 0), stop=(kh == NKH - 1))
                xts = xtp.tile([P, NFT, B], f32, name="xts")
                nc.sync.dma_start(
                    xts[:], d_xT[:, :, t * B:(t + 1) * B].rearrange("a p b -> p a b"))
                eo = eop.tile([P, NFT, B], f32, name="eo")
                for ft in range(NFT):
                    tmp = pwp.tile([P, B], f32, name="ptmp")
                    nc.vector.tensor_scalar_add(tmp[:], pj[:, ft * B:(ft + 1) * B],
                                                sb_outb[:, ft:ft + 1])
                    nc.vector.tensor_add(eo[:, ft, :], tmp[:], xts[:, ft, :])
                nc.sync.dma_start(d_enc[t].rearrange("f p b -> p f b"), eo[:])
                return eo

            # ---- encoder ----
            for t in range(n_enc):
                psA = pg.tile([P, 2 * B], f32, name="g0a")
                psB = pg.tile([P, 2 * B], f32, name="g0b")
                for g in range(NG):
                    ps = (psA, psB)[g // 2][:, (g % 2) * B:(g % 2) * B + B]
                    for kh in range(NKH):
                        nc.tensor.matmul(ps, w_hh0[:, kh, g, :], h0g[:, kh, :],
                                         start=(kh == 0), stop=(kh == NKH - 1))
                if t > 0:
                    emit_enc_proj(t - 1)
                xw = xwp.tile([P, NG, B], f32, name="xw")
                nc.sync.dma_start(xw[:], d_xw0[t].rearrange("g p b -> p g b"))
                h0send = hsp.tile([P, B], bf, name="h0send")
                gate_pointwise(psA, psB, sb_b0, c0, h0send, pre_add=xw)
                allgather(h0send, d_cc0i, d_cc0o, h0g)

                psC = pg2.tile([P, 2 * B], f32, name="g1a")
                psD = pg2.tile([P, 2 * B], f32, name="g1b")
                for g in range(NG):
                    ps = (psC, psD)[g // 2][:, (g % 2) * B:(g % 2) * B + B]
                    for kh in range(NKH):
                        nc.tensor.matmul(ps, w_hh1[:, kh, g, :], h1g[:, kh, :],
                                         start=(kh == 0), stop=False)
                    for kh in range(NKH):
                        nc.tensor.matmul(ps, w_ih1[:, kh, g, :], h0g[:, kh, :],
                                         start=False, stop=(kh == NKH - 1))
                h1send = hsp.tile([P, B], bf, name="h1send")
                gate_pointwise(psC, psD, sb_b1, c1, h1send)
                allgather(h1send, d_cc1i, d_cc1o, h1g)

            eo_last = emit_enc_proj(n_enc - 1)
            lastT = ltp.tile([P, NFT, B], f32r, name="lastT")
            nc.gpsimd.dma_start(lastT[:], eo_last[:])

            def emit_dec_proj(j):
                """new(j) = last + outWd @ h1(j) + cdec; updates lastT, writes dec_out[j]."""
                nonlocal lastT
                pj = pg3.tile([P, NFT * B], f32, name="pj")
                for ft in range(NFT):
                    for kh in range(NKH):
                        nc.tensor.matmul(pj[:, ft * B:(ft + 1) * B],
                                         w_outd[:, kh, ft, :], h1g[:, kh, :],
                                         start=(kh == 0), stop=(kh == NKH - 1))
                newt = ntp.tile([P, NFT, B], f32, name="newt")
                for ft in range(NFT):
                    tmp = pwp.tile([P, B], f32, name="ptmp")
                    nc.vector.tensor_scalar_add(tmp[:], pj[:, ft * B:(ft + 1) * B],
                                                sb_cdec[:, ft:ft + 1])
                    nc.vector.tensor_add(newt[:, ft, :], tmp[:],
                                         lastT[:, ft, :].bitcast(f32))
                nc.sync.dma_start(d_dec[j].rearrange("f p b -> p f b"), newt[:])
                lt_new = ltp.tile([P, NFT, B], f32r, name="lastT")
                nc.gpsimd.dma_start(lt_new[:], newt[:])
                lastT = lt_new

            # ---- decoder (software-pipelined) ----
            # gates0(j+1)'s Whh0 part issues before proj(j) so it fills the
            # AG(h1,j) wait; Wih0@lastT(j) completes the psum groups after the
            # projection updates lastT. One gate per PSUM bank keeps at most
            # one pending accumulation group per zeroing region.
            g0names = ["g0a", "g0b", "g0c", "g0d"]

            def g0_alloc():
                return [pg.tile([P, 2 * B], f32, name=g0names[g])
                        for g in range(NG)]

            def g0_hh_part(pss):
                for g in range(NG):
                    for kh in range(NKH):
                        nc.tensor.matmul(pss[g][:, :B], w_hh0[:, kh, g, :],
                                         h0g[:, kh, :],
                                         start=(kh == 0), stop=False)

            def g0_ih_part(pss):
                for g in range(NG):
                    for kf in range(NKF):
                        nc.tensor.matmul(pss[g][:, :B], w_ih0[:, kf, g, :],
                                         lastT[:, kf, :],
                                         start=False, stop=(kf == NKF - 1))

            def gate_pointwise4(pss, bias_sb, cstate, hout):
                acts = []
                for g, fn in ((0, ACT_SIG), (1, ACT_SIG), (2, ACT_TANH),
                              (3, ACT_SIG)):
                    a = pwp.tile([P, B], f32, name=f"act{g}")
                    nc.scalar.activation(a[:], pss[g][:, :B], fn,
                                         bias=bias_sb[:, g:g + 1])
                    acts.append(a)
                si, sf, tg, so_ = acts
                t1 = pwp.tile([P, B], f32, name="t1")
                t2 = pwp.tile([P, B], f32, name="t2")
                nc.vector.tensor_mul(t1[:], sf[:], cstate[:])
                nc.vector.tensor_mul(t2[:], si[:], tg[:])
                nc.vector.tensor_add(cstate[:], t1[:], t2[:])
                tcs = pwp.tile([P, B], f32, name="tc")
                nc.scalar.activation(tcs[:], cstate[:], ACT_TANH)
                nc.vector.tensor_mul(hout[:], so_[:], tcs[:])

            pss = g0_alloc()
            g0_hh_part(pss)
            g0_ih_part(pss)
            for j in range(n_dec):
                h0send = hsp.tile([P, B], bf, name="h0send")
                gate_pointwise4(pss, sb_b0, c0, h0send)
                allgather(h0send, d_cc0i, d_cc0o, h0g)

                psC = pg2.tile([P, 2 * B], f32, name="g1a")
                psD = pg2.tile([P, 2 * B], f32, name="g1b")
                for g in range(NG):
                    ps = (psC, psD)[g // 2][:, (g % 2) * B:(g % 2) * B + B]
                    for kh in range(NKH):
                        nc.tensor.matmul(ps, w_hh1[:, kh, g, :], h1g[:, kh, :],
                                         start=(kh == 0), stop=False)
                    for kh in range(NKH):
                        nc.tensor.matmul(ps, w_ih1[:, kh, g, :], h0g[:, kh, :],
                                         start=False, stop=(kh == NKH - 1))
                h1send = hsp.tile([P, B], bf, name="h1send")
                gate_pointwise(psC, psD, sb_b1, c1, h1send)
                allgather(h1send, d_cc1i, d_cc1o, h1g)

                if j < n_dec - 1:
                    pss = g0_alloc()
                    g0_hh_part(pss)        # only needs h0g(j): fills AG(h1,j)
                emit_dec_proj(j)           # needs h1g(j); updates lastT(j)
                if j < n_dec - 1:
                    g0_ih_part(pss)        # completes gates0(j+1)

    nc.compile()
    return nc


def _prep_inputs(x, W_ih0, W_hh0, b0, W_ih1, W_hh1, b1, out_W, out_b, dy_mu, dy_std,
                 n_enc):
    """Host-side transposes/shards. Returns list of per-core input dicts."""
    import ml_dtypes
    bfnp = ml_dtypes.bfloat16
    xT = np.ascontiguousarray(x[:, :n_enc].transpose(2, 1, 0)).reshape(NKF, P, n_enc * B)

    def gate_slices(W, k, nk):
        # W [4H, K] -> per-core slice, laid out [P, nk, NG, P]
        w4 = W.reshape(NG, H, W.shape[1])[:, k * P:(k + 1) * P, :]     # [NG, P(m), K]
        return np.ascontiguousarray(
            w4.reshape(NG, P, nk, P).transpose(3, 2, 0, 1))            # [P, nk, NG, P]

    out_Wd = out_W * dy_std[:, None]
    outwt = np.ascontiguousarray(
        out_W.reshape(NFT, P, NKH, P).transpose(3, 2, 0, 1)).astype(bfnp)
    outwtd = np.ascontiguousarray(
        out_Wd.reshape(NFT, P, NKH, P).transpose(3, 2, 0, 1)).astype(bfnp)
    outbt = np.ascontiguousarray(out_b.reshape(NFT, P).T)
    cdect = np.ascontiguousarray((dy_std * out_b + dy_mu).reshape(NFT, P).T)

    in_maps = []
    for k in range(NCORES):
        b0t = np.ascontiguousarray(b0.reshape(NG, H)[:, k * P:(k + 1) * P].T)
        b1t = np.ascontiguousarray(b1.reshape(NG, H)[:, k * P:(k + 1) * P].T)
        in_maps.append({
            "xT": xT,
            "wih0t": gate_slices(W_ih0, k, NKF),
            "whh0t": gate_slices(W_hh0, k, NKH).astype(bfnp),
            "wih1t": gate_slices(W_ih1, k, NKH).astype(bfnp),
            "whh1t": gate_slices(W_hh1, k, NKH).astype(bfnp),
            "outwt": outwt, "outwtd": outwtd,
            "b0t": b0t, "b1t": b1t, "outbt": outbt, "cdect": cdect,
        })
    return in_maps


def run_device(x, W_ih0, W_hh0, b0, W_ih1, W_hh1, b1, out_W, out_b, dy_mu, dy_std,
               n_enc, n_dec):
    """Run the Bass kernel; returns (enc_out, dec_out) from core 0."""
    from concourse.bass_utils import run_bass_kernel_spmd
    key = (n_enc, n_dec)
    if key not in _CACHE:
        _CACHE[key] = _build(n_enc, n_dec)
    nc = _CACHE[key]
    in_maps = _prep_inputs(x, W_ih0, W_hh0, b0, W_ih1, W_hh1, b1, out_W, out_b,
                           dy_mu, dy_std, n_enc)
    res = run_bass_kernel_spmd(nc, in_maps, core_ids=list(range(NCORES)), trace=False)
    out = res.results[0]
    return out["enc_out"], out["dec_out"]


def kernel(**inputs):
    x = np.asarray(inputs["x"], np.float32)
    t = int(np.asarray(inputs["t"]))
    args = [np.asarray(inputs[k], np.float32) for k in
            ["W_ih0", "W_hh0", "b0", "W_ih1", "W_hh1", "b1",
             "out_W", "out_b", "dy_mu", "dy_std"]]
    n_enc, n_dec = x.shape[1], t - 1
    enc, dec = run_device(x, *args, n_enc, n_dec)
    # enc [S, NFT, P, B] -> [B, S, F]; same for dec
    enc_b = np.ascontiguousarray(enc.transpose(3, 0, 1, 2)).reshape(B, n_enc, F)
    dec_b = np.ascontiguousarray(dec.transpose(3, 0, 1, 2)).reshape(B, n_dec, F)
    return np.concatenate([x[:, :1], enc_b, dec_b], axis=1)
